# revision 12
# baseline (speedup 1.0000x reference)
"""Trainium2 Bass kernel for GNN message passing (SSIM-weighted edge aggregation).

Problem (per batch element, one NeuronCore each; B=8 across 8 cores):
  x, xp: [C=96, N=3136];  edge_index: idx_i/idx_j [N, K=18] node ids.
  For each (n, k): gather channel columns x_i = x[:, idx_i], x_j = x[:, idx_j],
  compute SSIM-like scalar sff(n,k) from channel stats, output
    Ex[c, n] = sum_k |xp_i - xp_j| * sff + sum_k xp_i + sum_k xp_j.

Device strategy (v3 — convoy-free software pipeline):
  0. HOST builds the DRAM gather table [N, 256] bf16 rows
     [x.T(96) | xp.T(96) | mu | var | pad] (512B rows = full-rate DMA
     descriptors) plus per-chunk wrapped int16 index layouts.  No on-device
     table-build phase.
  1. Gathers are issued TWO chunks ahead on alternating SWDGE queue pairs
     ({0,1} even chunks, {2,3} odd) so descriptor generation for chunk c+2
     never waits behind chunk c+1's drain.
  2. gt's readers are all EARLY: xp-sum matmuls for chunk c+1 are issued one
     iteration ahead (its gather already landed), and on DVE only P = x_i*x_j,
     D = xp_i-xp_j and a compact mu/var copy touch gt.  The WAR release for
     the next gather therefore never rides a long dependency chain.
  3. Channel-dot via bf16 halving-tree (96->48->24->12->6) + small f32
     reduce; sff chain on compact [128, 36] f32 tiles; R = |D| * sff.
  4. K-reduction via TensorE one-hot matmuls accumulating in PSUM: xp sums
     start the po2 group (stop=False), R matmuls continue it same-iteration
     (skip_group_check) and stop.  Finals (PSUM copy + add + out DMA) are
     carried one iteration so no engine idles on them.
Host reassembles: out.T per core, stack -> [8, 96, 3136, 1].
"""

import sys

import numpy as np

sys.path.insert(0, "/opt/trn_rl_repo")

B, C, N, K = 8, 96, 3136, 18
C1 = 1e-6
C2 = 1e-6
ROW = 256  # table row bf16 elements (512B; dma_gather needs %256B==0)
CH2 = 256  # chunk: 256 output nodes
NCH2 = (N + CH2 - 1) // CH2  # 13 chunks (12 x 256 + 1 x 64)
MAXCOLS = 2 * CH2 * K // 16  # idx cols per chunk (576)
MU_COL = 192  # bf16 col of per-node channel mean
VAR_COL = 193  # bf16 col of per-node channel variance

_nc_cache = None


def _qsizes(nblocks):
    """Split nblocks gather blocks across the 4 SWDGE queues."""
    base, rem = divmod(nblocks, 4)
    return [base + (1 if q < rem else 0) for q in range(4)]


def _build_nc():
    import concourse.bacc as bacc
    import concourse.mybir as mybir
    import concourse.tile as tile
    from concourse.library_config import mlp
    from contextlib import ExitStack

    f32 = mybir.dt.float32
    bf16 = mybir.dt.bfloat16
    i16 = mybir.dt.int16
    AF = mybir.ActivationFunctionType
    OP = mybir.AluOpType
    AX = mybir.AxisListType

    nc = bacc.Bacc(None, target_bir_lowering=False, debug=False, num_swdge_queues=4)

    table = nc.dram_tensor("table", [N, ROW], bf16, kind="ExternalInput")
    idx_d = nc.dram_tensor("idx", [128, NCH2, MAXCOLS], i16, kind="ExternalInput")
    mb_d = nc.dram_tensor("mbase", [128, K, 128], bf16, kind="ExternalInput")
    out_d = nc.dram_tensor("out", [N, C], f32, kind="ExternalOutput")

    with ExitStack() as ctx:
        tc = ctx.enter_context(tile.TileContext(nc))
        const = ctx.enter_context(tc.tile_pool(name="const", bufs=1))
        gath = ctx.enter_context(tc.tile_pool(name="gath", bufs=2))
        xpcp = ctx.enter_context(tc.tile_pool(name="xpcp", bufs=2))
        work = ctx.enter_context(tc.tile_pool(name="work", bufs=2))
        tree = ctx.enter_context(tc.tile_pool(name="tree", bufs=2))
        stat = ctx.enter_context(tc.tile_pool(name="stat", bufs=2))
        outp = ctx.enter_context(tc.tile_pool(name="outp", bufs=3))
        psA = ctx.enter_context(tc.tile_pool(name="psA", bufs=3, space="PSUM"))

        nc.gpsimd.load_library(mlp)

        # idx for chunks 0-1 loads first so gather(0) starts ASAP; the rest
        # (and the one-hot weights) stream in behind it.
        idxA = const.tile([128, 2, MAXCOLS], i16)
        nc.sync.dma_start(out=idxA[:], in_=idx_d[:, 0:2, :])
        mb_sb = const.tile([128, K, 128], bf16)
        nc.sync.dma_start(out=mb_sb[:], in_=mb_d[:])
        idxB = const.tile([128, NCH2 - 2, MAXCOLS], i16)
        nc.sync.dma_start(out=idxB[:], in_=idx_d[:, 2:NCH2, :])

        def idx_sl(c, lo, hi):
            if c < 2:
                return idxA[:, c, lo:hi]
            return idxB[:, c - 2, lo:hi]

        def cdims(c):
            n0 = c * CH2
            nr = min(CH2, N - n0)
            nbs = nr * K // 128  # blocks per side (36 or 9)
            npo = (nbs + 17) // 18  # output 128-node groups (2 or 1)
            return n0, nr, nbs, npo

        gts = {}

        def issue_gather(c):
            _, _, nbs, _ = cdims(c)
            gt = gath.tile([128, 4 * K, ROW], bf16, tag="gt", name=f"gt_{c}")
            off_blk = 0
            off_col = 0
            for q, sz in enumerate(_qsizes(2 * nbs)):
                nio = sz * 128
                nc.gpsimd.dma_gather(
                    gt[:, off_blk : off_blk + sz, :],
                    table[:],
                    idx_sl(c, off_col, off_col + nio // 16),
                    nio,
                    nio,
                    ROW,
                    single_packet=False,
                    queue_num=q,
                )
                off_blk += sz
                off_col += nio // 16
            gts[c] = gt

        po2_all = {}

        def xp_mm(c, xpc):
            """Start the po2 PSUM group with the xp_i+xp_j one-hot sums.

            rhs comes from the XPC staging copy, NOT gt, so the tensor queue
            never holds up the gather buffer rotation.
            """
            _, _, nbs, npo = cdims(c)
            po2s = [
                psA.tile([128, 2, 96], f32, tag=f"po2{h}", name=f"po2{h}_{c}")
                for h in range(npo)
            ]
            for b in range(nbs):
                h, bb = divmod(b, 18)
                nc.tensor.matmul(
                    out=po2s[h][:, :, :], lhsT=mb_sb[:, bb, :],
                    rhs=xpc[:, b : 2 * nbs : nbs, :],
                    start=(bb == 0), stop=False,
                )
            po2_all[c] = po2s

        def finals(c):
            """PSUM -> SBUF -> DRAM for a chunk whose po2 group has closed."""
            n0, nr, _, npo = cdims(c)
            po2s = po2_all.pop(c)
            for h in range(npo):
                nh = min(128, nr - h * 128)
                t12 = outp.tile([128, 2, C], f32, tag="t12", name=f"t12_{c}_{h}")
                nc.scalar.activation(
                    out=t12[:nh, :, :], in_=po2s[h][:nh, :, :], func=AF.Copy
                )
                ot = outp.tile([128, C], f32, tag="ot", name=f"ot_{c}_{h}")
                nc.vector.tensor_add(
                    out=ot[:nh, :], in0=t12[:nh, 0, :], in1=t12[:nh, 1, :]
                )
                m0 = n0 + h * 128
                nc.sync.dma_start(out=out_d[m0 : m0 + nh, :], in_=ot[:nh, :])

        issue_gather(0)
        issue_gather(1)

        for c in range(NCH2):
            if c + 2 < NCH2:
                issue_gather(c + 2)
            gt = gts.pop(c)
            n0, nr, nbs, npo = cdims(c)

            x_i = gt[:, 0:nbs, 0:96]
            x_j = gt[:, nbs : 2 * nbs, 0:96]
            xp_i = gt[:, 0:nbs, 96:192]
            xp_j = gt[:, nbs : 2 * nbs, 96:192]

            # gt readers on DVE — all up front so the WAR for gather(c+2)
            # releases early.
            P = work.tile([128, 2 * K, 96], bf16, tag="P", name=f"P_{c}")
            nc.vector.tensor_mul(out=P[:, :nbs, :], in0=x_i, in1=x_j)
            D = work.tile([128, 2 * K, 96], bf16, tag="D", name=f"D_{c}")
            nc.vector.tensor_sub(out=D[:, :nbs, :], in0=xp_i, in1=xp_j)
            xpc = xpcp.tile([128, 4 * K, 96], bf16, tag="xpc", name=f"xpc_{c}")
            nc.vector.tensor_copy(
                out=xpc[:, : 2 * nbs, :], in_=gt[:, 0 : 2 * nbs, 96:192]
            )
            mvc = stat.tile([128, 4 * K, 2], f32, tag="mvc", name=f"mvc_{c}")
            nc.vector.tensor_copy(
                out=mvc[:, : 2 * nbs, :], in_=gt[:, 0 : 2 * nbs, MU_COL : MU_COL + 2]
            )

            xp_mm(c, xpc)
            # previous chunk's finals slot in here: every input long ready.
            if c >= 1:
                finals(c - 1)

            A = work.tile([128, 2 * K, 96], bf16, tag="A", name=f"A_{c}")
            nc.scalar.activation(out=A[:, :nbs, :], in_=D[:, :nbs, :], func=AF.Abs)

            # channel-dot: bf16 halving tree then small f32 reduce
            t48 = tree.tile([128, 2 * K, 48], bf16, tag="t48", name=f"t48_{c}")
            nc.vector.tensor_add(
                out=t48[:, :nbs, :], in0=P[:, :nbs, 0:48], in1=P[:, :nbs, 48:96]
            )
            t24 = tree.tile([128, 2 * K, 24], bf16, tag="t24", name=f"t24_{c}")
            nc.vector.tensor_add(
                out=t24[:, :nbs, :], in0=t48[:, :nbs, 0:24], in1=t48[:, :nbs, 24:48]
            )
            t12 = tree.tile([128, 2 * K, 12], bf16, tag="t12", name=f"t12_{c}")
            nc.vector.tensor_add(
                out=t12[:, :nbs, :], in0=t24[:, :nbs, 0:12], in1=t24[:, :nbs, 12:24]
            )
            t6 = tree.tile([128, 2 * K, 6], bf16, tag="t6", name=f"t6_{c}")
            nc.vector.tensor_add(
                out=t6[:, :nbs, :], in0=t12[:, :nbs, 0:6], in1=t12[:, :nbs, 6:12]
            )
            ps = stat.tile([128, 2 * K], f32, tag="ps", name=f"ps_{c}")
            nc.vector.tensor_reduce(
                out=ps[:, :nbs], in_=t6[:, :nbs, :], axis=AX.X, op=OP.add
            )

            # sff chain on compact [128, nbs] f32
            mu_i = mvc[:, 0:nbs, 0]
            mu_j = mvc[:, nbs : 2 * nbs, 0]
            var_i = mvc[:, 0:nbs, 1]
            var_j = mvc[:, nbs : 2 * nbs, 1]

            def st(tag, w=2 * K):
                return stat.tile([128, w], f32, tag=tag, name=f"{tag}_{c}")

            mm, t1, t2 = st("mm"), st("t1"), st("t2")
            nc.vector.tensor_mul(out=mm[:, :nbs], in0=mu_i, in1=mu_j)
            nc.vector.tensor_mul(out=t1[:, :nbs], in0=mu_i, in1=mu_i)
            nc.vector.tensor_mul(out=t2[:, :nbs], in0=mu_j, in1=mu_j)
            dd = st("dd", 4 * K)  # [den1 | den2]
            nc.vector.scalar_tensor_tensor(
                out=dd[:, 0:nbs], in0=t1[:, :nbs], scalar=C1, in1=t2[:, :nbs],
                op0=OP.add, op1=OP.add,
            )
            nc.vector.scalar_tensor_tensor(
                out=dd[:, 2 * K : 2 * K + nbs], in0=var_i, scalar=C2, in1=var_j,
                op0=OP.add, op1=OP.add,
            )
            rec = st("rec", 4 * K)
            nc.vector.reciprocal(out=rec[:, 0:nbs], in_=dd[:, 0:nbs])
            nc.vector.reciprocal(
                out=rec[:, 2 * K : 2 * K + nbs], in_=dd[:, 2 * K : 2 * K + nbs]
            )
            cv = st("cv")
            nc.vector.scalar_tensor_tensor(
                out=cv[:, :nbs], in0=ps[:, :nbs], scalar=1.0 / 96.0, in1=mm[:, :nbs],
                op0=OP.mult, op1=OP.subtract,
            )
            nn = st("nn", 4 * K)  # [num1 | num2]
            nc.vector.tensor_scalar(
                out=nn[:, 0:nbs], in0=mm[:, :nbs], scalar1=2.0, scalar2=C1,
                op0=OP.mult, op1=OP.add,
            )
            nc.vector.tensor_scalar(
                out=nn[:, 2 * K : 2 * K + nbs], in0=cv[:, :nbs], scalar1=2.0,
                scalar2=C2, op0=OP.mult, op1=OP.add,
            )
            SS = st("SS", 4 * K)
            nc.vector.tensor_mul(out=SS[:, 0:nbs], in0=nn[:, 0:nbs], in1=rec[:, 0:nbs])
            nc.vector.tensor_mul(
                out=SS[:, 2 * K : 2 * K + nbs], in0=nn[:, 2 * K : 2 * K + nbs],
                in1=rec[:, 2 * K : 2 * K + nbs],
            )
            s12, sff = st("s12"), st("sff")
            nc.vector.tensor_mul(
                out=s12[:, :nbs], in0=SS[:, 0:nbs], in1=SS[:, 2 * K : 2 * K + nbs]
            )
            nc.vector.tensor_scalar(
                out=sff[:, :nbs], in0=s12[:, :nbs], scalar1=-1.0, scalar2=1.0,
                op0=OP.mult, op1=OP.add,
            )

            R = work.tile([128, 2 * K, 96], bf16, tag="R", name=f"R_{c}")
            nc.vector.tensor_mul(
                out=R[:, :nbs, :],
                in0=A[:, :nbs, :],
                in1=sff[:, :nbs].to_broadcast((128, nbs, 96)),
            )

            # R matmuls continue (and close) the po2 group of THIS chunk
            po2s = po2_all[c]
            for b in range(nbs):
                h, bb = divmod(b, 18)
                nc.tensor.matmul(
                    out=po2s[h][:, 0, :], lhsT=mb_sb[:, bb, :], rhs=R[:, b, :],
                    start=False, stop=(bb == 17 or b == nbs - 1),
                    skip_group_check=True,
                )

        finals(NCH2 - 1)

    nc.compile()
    return nc


def _get_nc():
    global _nc_cache
    if _nc_cache is None:
        _nc_cache = _build_nc()
    return _nc_cache


def _build_idx(idx_i, idx_j):
    """idx_i/idx_j: [N, K] int -> [128, NCH2, MAXCOLS] int16 wrapped layout.

    Per chunk the 2*nr*K indices (i-side then j-side) are split into 4
    contiguous block-ranges (one per SWDGE queue), each independently wrapped
    into 16 partitions and replicated across the 8 Q7 core pairs.
    """
    chunks = []
    for c in range(NCH2):
        n0 = c * CH2
        n1 = min(n0 + CH2, N)
        comb = np.concatenate(
            [idx_i[n0:n1].reshape(-1), idx_j[n0:n1].reshape(-1)]
        ).astype(np.int16)
        nblocks = comb.size // 128
        cols = []
        off = 0
        for sz in _qsizes(nblocks):
            seg = comb[off * 128 : (off + sz) * 128]
            off += sz
            cols.append(seg.reshape(-1, 16).T)  # [16, sz*8]
        w = np.concatenate(cols, axis=1)
        full = np.tile(w, (8, 1))  # replicate across the 8 q7 cores
        if full.shape[1] < MAXCOLS:
            full = np.pad(full, ((0, 0), (0, MAXCOLS - full.shape[1])))
        chunks.append(full)
    return np.ascontiguousarray(np.stack(chunks, axis=1))


def _mbase():
    import ml_dtypes

    p = np.arange(128)[:, None, None]
    bb = np.arange(K)[None, :, None]
    m = np.arange(128)[None, None, :]
    mb = ((bb * 128 + p) // K == m).astype(np.float32)
    return np.ascontiguousarray(mb.astype(ml_dtypes.bfloat16))


def _build_table(xs, xps):
    """xs/xps: [C, N] f32 -> [N, ROW] bf16 table rows."""
    import ml_dtypes

    t = np.zeros((N, ROW), dtype=ml_dtypes.bfloat16)
    t[:, 0:C] = xs.T.astype(ml_dtypes.bfloat16)
    t[:, C : 2 * C] = xps.T.astype(ml_dtypes.bfloat16)
    mu = xs.mean(axis=0, dtype=np.float64)
    var = (xs.astype(np.float64) ** 2).mean(axis=0) - mu**2
    t[:, MU_COL] = mu.astype(ml_dtypes.bfloat16)
    t[:, VAR_COL] = var.astype(ml_dtypes.bfloat16)
    return np.ascontiguousarray(t)


def kernel(x, x_p, edge_index):
    from concourse.bass_utils import run_bass_kernel_spmd

    xs = np.ascontiguousarray(x[..., 0], dtype=np.float32)  # [B, C, N]
    xps = np.ascontiguousarray(x_p[..., 0], dtype=np.float32)
    idx_j_all = np.asarray(edge_index[0])  # neighbors
    idx_i_all = np.asarray(edge_index[1])  # centers
    mb = _mbase()

    in_maps = []
    for b in range(B):
        in_maps.append(
            {
                "table": _build_table(xs[b], xps[b]),
                "idx": _build_idx(idx_i_all[b], idx_j_all[b]),
                "mbase": mb,
            }
        )

    nc = _get_nc()
    res = run_bass_kernel_spmd(nc, in_maps, list(range(B))).results
    out = np.stack([r["out"].T for r in res])  # [B, C, N]
    return np.ascontiguousarray(out[..., None]).astype(np.float32)


if __name__ == "__main__":
    # quick smoke test with random data
    rng = np.random.default_rng(0)
    x = rng.standard_normal((B, C, N, 1), dtype=np.float32)
    x_p = rng.random((B, C, N, 1), dtype=np.float32)
    ei = rng.integers(0, N, size=(2, B, N, K)).astype(np.int32)
    out = kernel(x, x_p, ei)
    print(out.shape, out.dtype)


# revision 15
# speedup vs baseline: 1.0501x; 1.0501x over previous
"""Trainium2 Bass kernel for GNN message passing (SSIM-weighted edge aggregation).

Problem (per batch element, one NeuronCore each; B=8 across 8 cores):
  x, xp: [C=96, N=3136];  edge_index: idx_i/idx_j [N, K=18] node ids.
  For each (n, k): gather channel columns x_i = x[:, idx_i], x_j = x[:, idx_j],
  compute SSIM-like scalar sff(n,k) from channel stats, output
    Ex[c, n] = sum_k |xp_i - xp_j| * sff + sum_k xp_i + sum_k xp_j.

Device strategy (v3 — convoy-free software pipeline):
  0. HOST builds the DRAM gather table [N, 256] bf16 rows
     [x.T(96) | xp.T(96) | mu | var | pad] (512B rows = full-rate DMA
     descriptors) plus per-chunk wrapped int16 index layouts.  No on-device
     table-build phase.
  1. Gathers are issued TWO chunks ahead on alternating SWDGE queue pairs
     ({0,1} even chunks, {2,3} odd) so descriptor generation for chunk c+2
     never waits behind chunk c+1's drain.
  2. gt's readers are all EARLY: xp-sum matmuls for chunk c+1 are issued one
     iteration ahead (its gather already landed), and on DVE only P = x_i*x_j,
     D = xp_i-xp_j and a compact mu/var copy touch gt.  The WAR release for
     the next gather therefore never rides a long dependency chain.
  3. Channel-dot via bf16 halving-tree (96->48->24->12->6) + small f32
     reduce; sff chain on compact [128, 36] f32 tiles; R = |D| * sff.
  4. K-reduction via TensorE one-hot matmuls accumulating in PSUM: xp sums
     start the po2 group (stop=False), R matmuls continue it same-iteration
     (skip_group_check) and stop.  Finals (PSUM copy + add + out DMA) are
     carried one iteration so no engine idles on them.
Host reassembles: out.T per core, stack -> [8, 96, 3136, 1].
"""

import sys

import numpy as np

sys.path.insert(0, "/opt/trn_rl_repo")

B, C, N, K = 8, 96, 3136, 18
C1 = 1e-6
C2 = 1e-6
ROW = 256  # table row bf16 elements (512B; dma_gather needs %256B==0)
CH2 = 256  # chunk: 256 output nodes
NCH2 = (N + CH2 - 1) // CH2  # 13 chunks (12 x 256 + 1 x 64)
MAXCOLS = 2 * CH2 * K // 16  # idx cols per chunk (576)
MU_COL = 192  # bf16 col of per-node channel mean
VAR_COL = 193  # bf16 col of per-node channel variance

_nc_cache = None


def _qsizes(nblocks):
    """Split nblocks gather blocks across the 4 SWDGE queues."""
    base, rem = divmod(nblocks, 4)
    return [base + (1 if q < rem else 0) for q in range(4)]


def _build_nc():
    import concourse.bacc as bacc
    import concourse.mybir as mybir
    import concourse.tile as tile
    from concourse.library_config import mlp
    from contextlib import ExitStack

    f32 = mybir.dt.float32
    bf16 = mybir.dt.bfloat16
    i16 = mybir.dt.int16
    AF = mybir.ActivationFunctionType
    OP = mybir.AluOpType
    AX = mybir.AxisListType

    nc = bacc.Bacc(None, target_bir_lowering=False, debug=False, num_swdge_queues=4)

    table = nc.dram_tensor("table", [N, ROW], bf16, kind="ExternalInput")
    idx_d = nc.dram_tensor("idx", [128, NCH2, MAXCOLS], i16, kind="ExternalInput")
    mb_d = nc.dram_tensor("mbase", [128, K, 128], bf16, kind="ExternalInput")
    out_d = nc.dram_tensor("out", [N, C], f32, kind="ExternalOutput")

    with ExitStack() as ctx:
        tc = ctx.enter_context(tile.TileContext(nc))
        const = ctx.enter_context(tc.tile_pool(name="const", bufs=1))
        gath = ctx.enter_context(tc.tile_pool(name="gath", bufs=3))
        work = ctx.enter_context(tc.tile_pool(name="work", bufs=2))
        tree = ctx.enter_context(tc.tile_pool(name="tree", bufs=2))
        stat = ctx.enter_context(tc.tile_pool(name="stat", bufs=2))
        outp = ctx.enter_context(tc.tile_pool(name="outp", bufs=3))
        psA = ctx.enter_context(tc.tile_pool(name="psA", bufs=3, space="PSUM"))

        nc.gpsimd.load_library(mlp)

        # idx for chunks 0-1 loads first so gather(0) starts ASAP; the rest
        # (and the one-hot weights) stream in behind it.
        idxA = const.tile([128, 2, MAXCOLS], i16)
        nc.sync.dma_start(out=idxA[:], in_=idx_d[:, 0:2, :])
        mb_sb = const.tile([128, K, 128], bf16)
        nc.sync.dma_start(out=mb_sb[:], in_=mb_d[:])
        idxB = const.tile([128, NCH2 - 2, MAXCOLS], i16)
        nc.sync.dma_start(out=idxB[:], in_=idx_d[:, 2:NCH2, :])

        def idx_sl(c, lo, hi):
            if c < 2:
                return idxA[:, c, lo:hi]
            return idxB[:, c - 2, lo:hi]

        def cdims(c):
            n0 = c * CH2
            nr = min(CH2, N - n0)
            nbs = nr * K // 128  # blocks per side (36 or 9)
            npo = (nbs + 17) // 18  # output 128-node groups (2 or 1)
            return n0, nr, nbs, npo

        gts = {}

        def issue_gather(c):
            _, _, nbs, _ = cdims(c)
            gt = gath.tile([128, 4 * K, ROW], bf16, tag="gt", name=f"gt_{c}")
            off_blk = 0
            off_col = 0
            for q, sz in enumerate(_qsizes(2 * nbs)):
                nio = sz * 128
                nc.gpsimd.dma_gather(
                    gt[:, off_blk : off_blk + sz, :],
                    table[:],
                    idx_sl(c, off_col, off_col + nio // 16),
                    nio,
                    nio,
                    ROW,
                    single_packet=False,
                    queue_num=q,
                )
                off_blk += sz
                off_col += nio // 16
            gts[c] = gt

        po2_all = {}

        def xp_mm(c):
            """Start the po2 PSUM group with the xp_i+xp_j one-hot sums."""
            _, _, nbs, npo = cdims(c)
            gt = gts[c]
            po2s = [
                psA.tile([128, 2, 96], f32, tag=f"po2{h}", name=f"po2{h}_{c}")
                for h in range(npo)
            ]
            for b in range(nbs):
                h, bb = divmod(b, 18)
                nc.tensor.matmul(
                    out=po2s[h][:, :, :], lhsT=mb_sb[:, bb, :],
                    rhs=gt[:, b : 2 * nbs : nbs, 96:192],
                    start=(bb == 0), stop=False,
                )
            po2_all[c] = po2s

        def finals(c):
            """PSUM -> SBUF -> DRAM for a chunk whose po2 group has closed."""
            n0, nr, _, npo = cdims(c)
            po2s = po2_all.pop(c)
            for h in range(npo):
                nh = min(128, nr - h * 128)
                t12 = outp.tile([128, 2, C], f32, tag="t12", name=f"t12_{c}_{h}")
                nc.scalar.activation(
                    out=t12[:nh, :, :], in_=po2s[h][:nh, :, :], func=AF.Copy
                )
                ot = outp.tile([128, C], f32, tag="ot", name=f"ot_{c}_{h}")
                nc.vector.tensor_add(
                    out=ot[:nh, :], in0=t12[:nh, 0, :], in1=t12[:nh, 1, :]
                )
                m0 = n0 + h * 128
                nc.sync.dma_start(out=out_d[m0 : m0 + nh, :], in_=ot[:nh, :])

        issue_gather(0)
        issue_gather(1)
        xp_mm(0)

        for c in range(NCH2):
            if c + 2 < NCH2:
                issue_gather(c + 2)
            if c + 1 < NCH2:
                xp_mm(c + 1)
            gt = gts.pop(c)
            n0, nr, nbs, npo = cdims(c)

            x_i = gt[:, 0:nbs, 0:96]
            x_j = gt[:, nbs : 2 * nbs, 0:96]
            xp_i = gt[:, 0:nbs, 96:192]
            xp_j = gt[:, nbs : 2 * nbs, 96:192]

            # gt readers on DVE — all up front so the WAR for gather(c+2)
            # releases early.
            P = work.tile([128, 2 * K, 96], bf16, tag="P", name=f"P_{c}")
            nc.vector.tensor_mul(out=P[:, :nbs, :], in0=x_i, in1=x_j)
            D = work.tile([128, 2 * K, 96], bf16, tag="D", name=f"D_{c}")
            nc.vector.tensor_sub(out=D[:, :nbs, :], in0=xp_i, in1=xp_j)
            mvc = stat.tile([128, 4 * K, 2], f32, tag="mvc", name=f"mvc_{c}")
            nc.vector.tensor_copy(
                out=mvc[:, : 2 * nbs, :], in_=gt[:, 0 : 2 * nbs, MU_COL : MU_COL + 2]
            )

            # previous chunk's finals slot in here: every input long ready.
            if c >= 1:
                finals(c - 1)

            A = work.tile([128, 2 * K, 96], bf16, tag="A", name=f"A_{c}")
            nc.scalar.activation(out=A[:, :nbs, :], in_=D[:, :nbs, :], func=AF.Abs)

            # channel-dot: bf16 halving tree then small f32 reduce
            t48 = tree.tile([128, 2 * K, 48], bf16, tag="t48", name=f"t48_{c}")
            nc.vector.tensor_add(
                out=t48[:, :nbs, :], in0=P[:, :nbs, 0:48], in1=P[:, :nbs, 48:96]
            )
            t24 = tree.tile([128, 2 * K, 24], bf16, tag="t24", name=f"t24_{c}")
            nc.vector.tensor_add(
                out=t24[:, :nbs, :], in0=t48[:, :nbs, 0:24], in1=t48[:, :nbs, 24:48]
            )
            t12 = tree.tile([128, 2 * K, 12], bf16, tag="t12", name=f"t12_{c}")
            nc.vector.tensor_add(
                out=t12[:, :nbs, :], in0=t24[:, :nbs, 0:12], in1=t24[:, :nbs, 12:24]
            )
            t6 = tree.tile([128, 2 * K, 6], bf16, tag="t6", name=f"t6_{c}")
            nc.vector.tensor_add(
                out=t6[:, :nbs, :], in0=t12[:, :nbs, 0:6], in1=t12[:, :nbs, 6:12]
            )
            ps = stat.tile([128, 2 * K], f32, tag="ps", name=f"ps_{c}")
            nc.vector.tensor_reduce(
                out=ps[:, :nbs], in_=t6[:, :nbs, :], axis=AX.X, op=OP.add
            )

            # sff chain on compact [128, nbs] f32
            mu_i = mvc[:, 0:nbs, 0]
            mu_j = mvc[:, nbs : 2 * nbs, 0]
            var_i = mvc[:, 0:nbs, 1]
            var_j = mvc[:, nbs : 2 * nbs, 1]

            def st(tag, w=2 * K):
                return stat.tile([128, w], f32, tag=tag, name=f"{tag}_{c}")

            mm, t1, t2 = st("mm"), st("t1"), st("t2")
            nc.vector.tensor_mul(out=mm[:, :nbs], in0=mu_i, in1=mu_j)
            nc.vector.tensor_mul(out=t1[:, :nbs], in0=mu_i, in1=mu_i)
            nc.vector.tensor_mul(out=t2[:, :nbs], in0=mu_j, in1=mu_j)
            dd = st("dd", 4 * K)  # [den1 | den2]
            nc.vector.scalar_tensor_tensor(
                out=dd[:, 0:nbs], in0=t1[:, :nbs], scalar=C1, in1=t2[:, :nbs],
                op0=OP.add, op1=OP.add,
            )
            nc.vector.scalar_tensor_tensor(
                out=dd[:, 2 * K : 2 * K + nbs], in0=var_i, scalar=C2, in1=var_j,
                op0=OP.add, op1=OP.add,
            )
            rec = st("rec", 4 * K)
            nc.vector.reciprocal(out=rec[:, 0:nbs], in_=dd[:, 0:nbs])
            nc.vector.reciprocal(
                out=rec[:, 2 * K : 2 * K + nbs], in_=dd[:, 2 * K : 2 * K + nbs]
            )
            cv = st("cv")
            nc.vector.scalar_tensor_tensor(
                out=cv[:, :nbs], in0=ps[:, :nbs], scalar=1.0 / 96.0, in1=mm[:, :nbs],
                op0=OP.mult, op1=OP.subtract,
            )
            nn = st("nn", 4 * K)  # [num1 | num2]
            nc.vector.tensor_scalar(
                out=nn[:, 0:nbs], in0=mm[:, :nbs], scalar1=2.0, scalar2=C1,
                op0=OP.mult, op1=OP.add,
            )
            nc.vector.tensor_scalar(
                out=nn[:, 2 * K : 2 * K + nbs], in0=cv[:, :nbs], scalar1=2.0,
                scalar2=C2, op0=OP.mult, op1=OP.add,
            )
            SS = st("SS", 4 * K)
            nc.vector.tensor_mul(out=SS[:, 0:nbs], in0=nn[:, 0:nbs], in1=rec[:, 0:nbs])
            nc.vector.tensor_mul(
                out=SS[:, 2 * K : 2 * K + nbs], in0=nn[:, 2 * K : 2 * K + nbs],
                in1=rec[:, 2 * K : 2 * K + nbs],
            )
            s12, sff = st("s12"), st("sff")
            nc.vector.tensor_mul(
                out=s12[:, :nbs], in0=SS[:, 0:nbs], in1=SS[:, 2 * K : 2 * K + nbs]
            )
            nc.vector.tensor_scalar(
                out=sff[:, :nbs], in0=s12[:, :nbs], scalar1=-1.0, scalar2=1.0,
                op0=OP.mult, op1=OP.add,
            )

            R = work.tile([128, 2 * K, 96], bf16, tag="R", name=f"R_{c}")
            nc.vector.tensor_mul(
                out=R[:, :nbs, :],
                in0=A[:, :nbs, :],
                in1=sff[:, :nbs].to_broadcast((128, nbs, 96)),
            )

            # R matmuls continue (and close) the po2 group of THIS chunk
            po2s = po2_all[c]
            for b in range(nbs):
                h, bb = divmod(b, 18)
                nc.tensor.matmul(
                    out=po2s[h][:, 0, :], lhsT=mb_sb[:, bb, :], rhs=R[:, b, :],
                    start=False, stop=(bb == 17 or b == nbs - 1),
                    skip_group_check=True,
                )

        finals(NCH2 - 1)

    nc.compile()
    return nc


def _get_nc():
    global _nc_cache
    if _nc_cache is None:
        _nc_cache = _build_nc()
    return _nc_cache


def _build_idx(idx_i, idx_j):
    """idx_i/idx_j: [N, K] int -> [128, NCH2, MAXCOLS] int16 wrapped layout.

    Per chunk the 2*nr*K indices (i-side then j-side) are split into 4
    contiguous block-ranges (one per SWDGE queue), each independently wrapped
    into 16 partitions and replicated across the 8 Q7 core pairs.
    """
    chunks = []
    for c in range(NCH2):
        n0 = c * CH2
        n1 = min(n0 + CH2, N)
        comb = np.concatenate(
            [idx_i[n0:n1].reshape(-1), idx_j[n0:n1].reshape(-1)]
        ).astype(np.int16)
        nblocks = comb.size // 128
        cols = []
        off = 0
        for sz in _qsizes(nblocks):
            seg = comb[off * 128 : (off + sz) * 128]
            off += sz
            cols.append(seg.reshape(-1, 16).T)  # [16, sz*8]
        w = np.concatenate(cols, axis=1)
        full = np.tile(w, (8, 1))  # replicate across the 8 q7 cores
        if full.shape[1] < MAXCOLS:
            full = np.pad(full, ((0, 0), (0, MAXCOLS - full.shape[1])))
        chunks.append(full)
    return np.ascontiguousarray(np.stack(chunks, axis=1))


def _mbase():
    import ml_dtypes

    p = np.arange(128)[:, None, None]
    bb = np.arange(K)[None, :, None]
    m = np.arange(128)[None, None, :]
    mb = ((bb * 128 + p) // K == m).astype(np.float32)
    return np.ascontiguousarray(mb.astype(ml_dtypes.bfloat16))


def _build_table(xs, xps):
    """xs/xps: [C, N] f32 -> [N, ROW] bf16 table rows."""
    import ml_dtypes

    t = np.zeros((N, ROW), dtype=ml_dtypes.bfloat16)
    t[:, 0:C] = xs.T.astype(ml_dtypes.bfloat16)
    t[:, C : 2 * C] = xps.T.astype(ml_dtypes.bfloat16)
    mu = xs.mean(axis=0, dtype=np.float64)
    var = (xs.astype(np.float64) ** 2).mean(axis=0) - mu**2
    t[:, MU_COL] = mu.astype(ml_dtypes.bfloat16)
    t[:, VAR_COL] = var.astype(ml_dtypes.bfloat16)
    return np.ascontiguousarray(t)


def kernel(x, x_p, edge_index):
    from concourse.bass_utils import run_bass_kernel_spmd

    xs = np.ascontiguousarray(x[..., 0], dtype=np.float32)  # [B, C, N]
    xps = np.ascontiguousarray(x_p[..., 0], dtype=np.float32)
    idx_j_all = np.asarray(edge_index[0])  # neighbors
    idx_i_all = np.asarray(edge_index[1])  # centers
    mb = _mbase()

    in_maps = []
    for b in range(B):
        in_maps.append(
            {
                "table": _build_table(xs[b], xps[b]),
                "idx": _build_idx(idx_i_all[b], idx_j_all[b]),
                "mbase": mb,
            }
        )

    nc = _get_nc()
    res = run_bass_kernel_spmd(nc, in_maps, list(range(B))).results
    out = np.stack([r["out"].T for r in res])  # [B, C, N]
    return np.ascontiguousarray(out[..., None]).astype(np.float32)


if __name__ == "__main__":
    # quick smoke test with random data
    rng = np.random.default_rng(0)
    x = rng.standard_normal((B, C, N, 1), dtype=np.float32)
    x_p = rng.random((B, C, N, 1), dtype=np.float32)
    ei = rng.integers(0, N, size=(2, B, N, K)).astype(np.int32)
    out = kernel(x, x_p, ei)
    print(out.shape, out.dtype)
